# revision 4
# baseline (speedup 1.0000x reference)
"""Trainium2 Bass kernel: nearest-centroid assignment (vq_codebook).

Computes, for each row of `feats` [N, 512]:
    f = normalize([feats_n, 1])            (L2, with appended ones column)
    pred_n = labelset[argmin_l ||f - c_l||]   with c = initc[labelset]  [128, 513]

Equivalent argmax formulation used on device (monotone transform, per row n):
    argmin_l dist  ==  argmax_l  T[n, l]
    T[n, l] = sum_d feats[n,d] * c[l,d]  +  cdv_l  +  u_n * w2neg_l
    cdv_l   = c[l,512] - MBAR * w2c_l      (per-l constant, exact f32)
    u_n     = sqrt(|feats_n|^2 + 1) - MBAR
    w2neg_l = -(0.5*|c_l|^2 - mean)        (centered, fp16)

Sharding: pure data-parallel over rows, N/8 = 32768 rows per NeuronCore.

v2 dataflow: host pre-packs feats in fp16 *transposed* layout
    ftpack[g, p, k*512 + n] = feats[g*512 + n, k*128 + p]
so the DMA streams matmul-ready fT tiles (contiguous 4 KB per partition
line) and no on-device transposition of feats is needed at all. The
per-row norm term u is also host-computed (O(N*D) preprocessing, same
class as the centroid cpack) and shipped as a [1, 32768] fp16 row.
HBM traffic per core: 32 MiB fp16 (vs 64 MiB f32) -> ~93 us DMA floor.

Per-core dataflow (64 groups of 512 rows):
    DMA    : 1 MiB fT tile per 2 groups (fp16, natural partition layout)
    PE     : 4x matmul  G[l,n] += cT16_k.T @ fT_k      (fp16, N=512)
             1x matmul  G[l,n] += w2neg16.T @ u_row     (K=1 rank-1)
    ACT    : G_ps -> SBUF copy with bias=cdv (exact f32 per-l constant)
    PE     : 4x transpose 128x128 f32: G -> T[n, l]
    DVE    : max / max_index per 128-row subtile directly from PSUM

Scores stay exact fp32 from PSUM accumulation through argmax; the only
fp16 roundings are feats/c (as in v1) and the tiny u*w2c cross term.

Notes on this walrus build:
  - one sync wait per engine instruction: residual multi-wait
    instructions are split via same-engine NoOps (_split_multiwait).
  - all matmuls in the G accumulation group are fp16 (mixing fp16 and
    fp32r in one PSUM group silently corrupts on hardware).
"""

import os
import sys

import numpy as np

for _p in ("/opt/trn_rl_repo",):
    if _p not in sys.path and os.path.isdir(_p):
        sys.path.insert(0, _p)

import concourse.bass as bass
import concourse.mybir as mybir
import concourse.tile as tile
from concourse.bass_utils import run_bass_kernel_spmd

N, D, K = 262144, 512, 128
N_CORES = 8
ROWS_PER_CORE = N // N_CORES  # 32768
GROUP = 512  # rows per compute group (4 subtiles of 128)
DMA_GROUPS = 2  # groups per DMA (1 MiB transfers)

F32 = mybir.dt.float32
F16 = mybir.dt.float16
U32 = mybir.dt.uint32
AF = mybir.ActivationFunctionType

# fp16 constant pack [128, CPK16_W]
P16_CT = 0         # [128, 512] cT: cT[p, 128k+l] = c[l, 128k+p]
P16_W2 = 512       # row 0, [1, 128] w2neg = -(w2 - mean(w2))
CPK16_W = 640

# f32 constant pack [128, CPK32_W]
P32_IDENT = 0      # [128, 128] identity
P32_CDV = 128      # [128, 1] column: cdv_l = cd_l - MBAR*w2c_l
CPK32_W = 130

MBAR = float(np.sqrt(513.0))  # fixed shift for m; argmax-invariant


def _split_multiwait(nc):
    """Walrus (this build) allows one sync wait per engine instruction.

    Tile occasionally emits 2+ (data dep + buffer-slot release on another
    semaphore). Splitting is semantics-preserving: a same-engine NoOp placed
    immediately before the instruction carries the surplus waits; the engine
    executes in order, so all waits are still enforced before the instruction
    runs.
    """
    import bass_rust

    for fn in nc.m.functions:
        for blk in fn.blocks:
            out = []
            changed = False
            for inst in blk.instructions:
                si = getattr(inst, "sync_info", None)
                waits = list(si.on_wait) if si is not None else []
                if len(waits) > 1:
                    for w in waits[:-1]:
                        nop = mybir.InstNoOp(
                            name=nc.get_next_instruction_name(), ins=[], outs=[]
                        )
                        nop.engine = inst.engine
                        nop.sync_info = bass_rust.SyncInfo(
                            on_wait=[w], on_update=[]
                        )
                        out.append(nop)
                    inst.sync_info = bass_rust.SyncInfo(
                        on_wait=waits[-1:], on_update=list(si.on_update)
                    )
                    changed = True
                out.append(inst)
            if changed:
                blk.instructions = out


def build_core_program(rows=ROWS_PER_CORE, split_waits=True):
    """Bass program for one NeuronCore processing `rows` rows of feats."""
    assert rows % (GROUP * DMA_GROUPS) == 0
    ngroups = rows // GROUP           # 64
    ndma = ngroups // DMA_GROUPS      # 32
    ntiles = rows // 128              # 256

    nc = bass.Bass()
    ft_d = nc.declare_dram_parameter("ftpack", [ngroups, 128, 2048], F16, isOutput=False)
    cpk16_d = nc.declare_dram_parameter("cpk16", [128, CPK16_W], F16, isOutput=False)
    cpk32_d = nc.declare_dram_parameter("cpk32", [128, CPK32_W], F32, isOutput=False)
    u_d = nc.declare_dram_parameter("upack", [1, rows], F16, isOutput=False)
    idx_d = nc.declare_dram_parameter("idx8", [128, ntiles, 8], U32, isOutput=True)

    with tile.TileContext(nc) as tc:
        with (
            tc.tile_pool(name="const", bufs=1) as constp,
            tc.tile_pool(name="fin", bufs=3) as finp,
            tc.tile_pool(name="gs", bufs=3) as gsp,
            tc.tile_pool(name="small", bufs=8) as smallp,
            tc.tile_pool(name="outp", bufs=1) as outp,
            tc.tile_pool(name="gp", bufs=2, space="PSUM") as gpp,
            tc.tile_pool(name="tp", bufs=2, space="PSUM") as tpp,
            tc.tile_pool(name="warm", bufs=1, space="PSUM") as warmp,
        ):
            urow = constp.tile([1, rows], F16)
            nc.sync.dma_start(urow[:], u_d[:])
            cpk16 = constp.tile([128, CPK16_W], F16)
            nc.sync.dma_start(cpk16[:], cpk16_d[:])
            cpk32 = constp.tile([128, CPK32_W], F32)
            nc.sync.dma_start(cpk32[:], cpk32_d[:])

            cT16 = cpk16[:, P16_CT : P16_CT + 512]
            w2neg16_row = cpk16[0:1, P16_W2 : P16_W2 + 128]
            ident32 = cpk32[:, P32_IDENT : P32_IDENT + 128]
            cdv_col = cpk32[:, P32_CDV : P32_CDV + 1]
            idxacc = outp.tile([128, ntiles, 8], U32)

            # warmup: make each engine observe a const-DMA lane with a
            # single-wait instruction, so later instructions carry at most
            # one new wait each.
            warm_ps = warmp.tile([128, 128], F32, tag="warm_ps")
            nc.tensor.transpose(warm_ps[:], ident32, ident32)
            warm16_ps = warmp.tile([1, 128], F16, tag="warm16_ps")
            nc.tensor.transpose(
                warm16_ps[:], cpk16[:, 0:1], cpk16[:, 0:128]
            )
            act_warm = smallp.tile([1, 1], F32, tag="act_warm")
            nc.scalar.copy(act_warm[:], cpk32[0:1, 0:1])
            dve_warm = smallp.tile([1, 1], F16, tag="dve_warm")
            nc.vector.tensor_copy(dve_warm[:], urow[0:1, 0:1])

            for dg in range(ndma):
                ft = finp.tile([128, DMA_GROUPS, 4, 512], F16)
                src = ft_d[dg * DMA_GROUPS : (dg + 1) * DMA_GROUPS].rearrange(
                    "g p x -> p g x"
                )
                nc.gpsimd.dma_start(ft[:], src)

                for h in range(DMA_GROUPS):
                    g = dg * DMA_GROUPS + h
                    G_ps = gpp.tile([128, 512], F32)
                    for k in range(4):
                        nc.tensor.matmul(
                            G_ps[:],
                            cT16[:, k * 128 : (k + 1) * 128],
                            ft[:, h, k, :],
                            start=(k == 0),
                            stop=False,
                        )
                    nc.tensor.matmul(
                        G_ps[:],
                        w2neg16_row,
                        urow[0:1, g * 512 : (g + 1) * 512],
                        start=False,
                        stop=True,
                    )
                    # PSUM -> SBUF with exact f32 per-l constant folded in
                    Gs = gsp.tile([128, 512], F32)
                    nc.scalar.activation(
                        Gs[:], G_ps[:], AF.Identity, bias=cdv_col, scale=1.0
                    )
                    T_ps = tpp.tile([128, 512], F32)
                    for j in range(4):
                        nc.tensor.transpose(
                            T_ps[:, j * 128 : (j + 1) * 128],
                            Gs[:, j * 128 : (j + 1) * 128],
                            ident32,
                        )
                    for j in range(4):
                        mx = smallp.tile([128, 8], F32, tag="mx")
                        nc.vector.max(mx[:], T_ps[:, j * 128 : (j + 1) * 128])
                        nc.vector.max_index(
                            idxacc[:, g * 4 + j, :],
                            mx[:],
                            T_ps[:, j * 128 : (j + 1) * 128],
                        )

            nc.sync.dma_start(idx_d[:], idxacc[:])
    if split_waits:
        _split_multiwait(nc)
    return nc


def make_const_inputs(initc, labelset):
    c = np.asarray(initc, dtype=np.float32)[np.asarray(labelset).astype(np.int64)]
    assert c.shape == (K, D + 1)
    w2 = 0.5 * np.sum(c.astype(np.float64) ** 2, axis=1)
    # Centering w2 (and shifting m by MBAR) adds per-row constants to the
    # scores (argmax invariant) while keeping the fp16-rounded rank-1 term
    # u*w2c small. The per-l constant cdv ships exact as an f32 ACT bias.
    w2c = w2 - w2.mean()
    cdv = (c[:, D].astype(np.float64) - MBAR * w2c).astype(np.float32)

    cpk16 = np.zeros((128, CPK16_W), np.float16)
    for k in range(4):
        cpk16[:, P16_CT + k * 128 : P16_CT + (k + 1) * 128] = (
            c[:, k * 128 : (k + 1) * 128].T.astype(np.float16)
        )
    cpk16[0, P16_W2 : P16_W2 + 128] = (-w2c).astype(np.float16)

    cpk32 = np.zeros((128, CPK32_W), np.float32)
    cpk32[:, P32_IDENT : P32_IDENT + 128] = np.eye(128, dtype=np.float32)
    cpk32[:, P32_CDV] = cdv
    return {"cpk16": cpk16, "cpk32": cpk32}


def pack_feats_core(shard16):
    """[32768, 512] fp16 -> [64, 128, 2048] with P[g,p,k*512+n] = s[g*512+n, k*128+p]."""
    ngroups = shard16.shape[0] // GROUP
    P = shard16.reshape(ngroups, GROUP, 4, 128).transpose(0, 3, 2, 1)
    return np.ascontiguousarray(P).reshape(ngroups, 128, 2048)


def build_in_maps(feats, initc, labelset):
    feats = np.asarray(feats, dtype=np.float32)
    consts = make_const_inputs(initc, labelset)

    feats16 = feats.astype(np.float16)
    q = np.einsum("nd,nd->n", feats, feats, dtype=np.float64)
    u16 = (np.sqrt(q + 1.0) - MBAR).astype(np.float16)

    in_maps = []
    for core in range(N_CORES):
        sl = slice(core * ROWS_PER_CORE, (core + 1) * ROWS_PER_CORE)
        in_maps.append(
            {
                "ftpack": pack_feats_core(feats16[sl]),
                "upack": np.ascontiguousarray(u16[sl].reshape(1, -1)),
                **consts,
            }
        )
    return in_maps


def kernel(feats, initc, labelset):
    labelset_np = np.asarray(labelset)
    in_maps = build_in_maps(feats, initc, labelset)
    nc = build_core_program(ROWS_PER_CORE)
    res = run_bass_kernel_spmd(nc, in_maps, list(range(N_CORES)))

    preds = []
    for core in range(N_CORES):
        idx8 = np.asarray(res.results[core]["idx8"])  # [128, ntiles, 8]
        idx = idx8[:, :, 0].T.reshape(-1).astype(np.int64)  # row 128*t+p
        preds.append(idx)
    idx_all = np.concatenate(preds)
    return labelset_np[idx_all]


# revision 10
# speedup vs baseline: 1.3339x; 1.3339x over previous
"""Trainium2 Bass kernel: nearest-centroid assignment (vq_codebook).

Computes, for each row of `feats` [N, 512]:
    f = normalize([feats_n, 1])            (L2, with appended ones column)
    pred_n = labelset[argmin_l ||f - c_l||]   with c = initc[labelset]  [128, 513]

Equivalent argmax formulation used on device (monotone transform, per row n):
    argmin_l dist  ==  argmax_l  T[n, l]
    T[n, l] = sum_d feats[n,d] * c[l,d]  +  cdv_l  +  u_n * w2neg_l
    cdv_l   = c[l,512] - MBAR * w2c_l      (per-l constant, exact f32)
    u_n     = sqrt(|feats_n|^2 + 1) - MBAR
    w2neg_l = -(0.5*|c_l|^2 - mean)        (centered, fp16)

Sharding: pure data-parallel over rows, N/8 = 32768 rows per NeuronCore.

v3 dataflow: host pre-packs feats in fp16 *transposed* layout
    ftpack[p, g, k*512 + n] = feats[g*512 + n, k*128 + p]
so the DMA streams matmul-ready fT tiles (contiguous 8+ KB per partition
line per transfer); no on-device transposition of feats is needed. The
per-row norm term u is host-computed (O(N*D) preprocessing, same class
as the centroid cpack) and shipped as a [1, 32768] fp16 row. HBM
traffic per core: 32 MiB fp16.

Argmax via u32 score|index encoding (avoids the 3-instruction
MAX8/MATCH_VALUE_LOAD/FIND_INDEX8 chain per 128-row subtile):
    scores shifted positive (+4096 folded into the ACT bias), so their
    f32 bit patterns order like the values as unsigned ints; GPSIMD
    clears the low 7 mantissa bits (quantization ~0.06 abs, far below
    typical top-2 gaps) and ORs in (127 - l) -- both per-partition
    scalars in G-layout where partition == l. PE transposes the encoded
    bits (pass-through for normal floats); a single MAX8 per subtile
    then yields max-and-argmax in one op; host decodes l = 127 - (v&127).
    Ties resolve to the smallest l, matching argmin's first-index rule.

Per-core dataflow (64 groups of 512 rows):
    DMA    : 1 MiB fT tile per 2 groups, alternating gpsimd/sync queues
    PE     : 4x matmul  G[l,n] += cT16_k.T @ fT_k      (fp16, N=512)
             1x matmul  G[l,n] += w2neg16.T @ u_row     (K=1 rank-1)
    ACT    : G_ps -> SBUF copy with bias = cdv + 4096  (exact f32)
    GPSIMD : encode S = (Gs & 0xFFFFFF80) | (127-l)     (tensor_scalar)
    PE     : 4x transpose 128x128 of encoded bits: S -> T_enc[n, l]
    DVE    : 1x MAX8 per subtile directly from PSUM -> idxacc

Notes on this walrus build:
  - one sync wait per engine instruction: residual multi-wait
    instructions are split via same-engine NoOps (_split_multiwait).
  - all matmuls in the G accumulation group are fp16 (mixing fp16 and
    fp32r in one PSUM group silently corrupts on hardware).
"""

import os
import sys

import numpy as np

for _p in ("/opt/trn_rl_repo",):
    if _p not in sys.path and os.path.isdir(_p):
        sys.path.insert(0, _p)

import concourse.bass as bass
import concourse.mybir as mybir
import concourse.tile as tile
from concourse.bass_utils import run_bass_kernel_spmd

N, D, K = 262144, 512, 128
N_CORES = 8
ROWS_PER_CORE = N // N_CORES  # 32768
GROUP = 512  # rows per compute group (4 subtiles of 128)
DMA_GROUPS = 2  # groups per DMA (1 MiB transfers)

F32 = mybir.dt.float32
F16 = mybir.dt.float16
U32 = mybir.dt.uint32
AF = mybir.ActivationFunctionType
ALU = mybir.AluOpType

# fp16 constant pack [128, CPK16_W]
P16_CT = 0         # [128, 512] cT: cT[p, 128k+l] = c[l, 128k+p]
P16_W2 = 512       # row 0, [1, 128] w2neg = -(w2 - mean(w2))
CPK16_W = 640

# f32 constant pack [128, CPK32_W]
P32_IDENT = 0      # [128, 128] identity
P32_CDV = 128      # [128, 1] column: cdv_l + SHIFT
CPK32_W = 130

# u32 constant pack [128, 2]: col 0 = mantissa mask, col 1 = 127 - l
LP_MASK = 0
LP_LIDX = 1

MBAR = float(np.sqrt(513.0))  # fixed shift for m; argmax-invariant
SHIFT = 4096.0                # makes scores positive (u32-orderable bits)
ENC_MASK = 0xFFFFFF80         # clear low 7 mantissa bits for the l index


def _split_multiwait(nc):
    """Walrus (this build) allows one sync wait per engine instruction.

    Tile occasionally emits 2+ (data dep + buffer-slot release on another
    semaphore). Splitting is semantics-preserving: a same-engine NoOp placed
    immediately before the instruction carries the surplus waits; the engine
    executes in order, so all waits are still enforced before the instruction
    runs.
    """
    import bass_rust

    for fn in nc.m.functions:
        for blk in fn.blocks:
            out = []
            changed = False
            for inst in blk.instructions:
                si = getattr(inst, "sync_info", None)
                waits = list(si.on_wait) if si is not None else []
                if len(waits) > 1:
                    for w in waits[:-1]:
                        nop = mybir.InstNoOp(
                            name=nc.get_next_instruction_name(), ins=[], outs=[]
                        )
                        nop.engine = inst.engine
                        nop.sync_info = bass_rust.SyncInfo(
                            on_wait=[w], on_update=[]
                        )
                        out.append(nop)
                    inst.sync_info = bass_rust.SyncInfo(
                        on_wait=waits[-1:], on_update=list(si.on_update)
                    )
                    changed = True
                out.append(inst)
            if changed:
                blk.instructions = out


def build_core_program(rows=ROWS_PER_CORE, split_waits=True):
    """Bass program for one NeuronCore processing `rows` rows of feats."""
    assert rows % (GROUP * DMA_GROUPS) == 0
    ngroups = rows // GROUP           # 64
    ndma = ngroups // DMA_GROUPS      # 32
    ntiles = rows // 128              # 256

    nc = bass.Bass()
    ft_d = nc.declare_dram_parameter("ftpack", [128, ngroups, 2048], F16, isOutput=False)
    cpk16_d = nc.declare_dram_parameter("cpk16", [128, CPK16_W], F16, isOutput=False)
    cpk32_d = nc.declare_dram_parameter("cpk32", [128, CPK32_W], F32, isOutput=False)
    lpack_d = nc.declare_dram_parameter("lpack", [128, 2], U32, isOutput=False)
    u_d = nc.declare_dram_parameter("upack", [1, rows], F16, isOutput=False)
    idx_d = nc.declare_dram_parameter("idx8", [128, ntiles, 8], U32, isOutput=True)

    with tile.TileContext(nc) as tc:
        with (
            tc.tile_pool(name="const", bufs=1) as constp,
            tc.tile_pool(name="fin", bufs=4) as finp,
            tc.tile_pool(name="gs", bufs=3) as gsp,
            tc.tile_pool(name="enc", bufs=3) as encp,
            tc.tile_pool(name="small", bufs=8) as smallp,
            tc.tile_pool(name="outp", bufs=1) as outp,
            tc.tile_pool(name="gp", bufs=3, space="PSUM") as gpp,
            tc.tile_pool(name="tp", bufs=3, space="PSUM") as tpp,
            tc.tile_pool(name="warm", bufs=1, space="PSUM") as warmp,
        ):
            cpk16 = constp.tile([128, CPK16_W], F16)
            nc.sync.dma_start(cpk16[:], cpk16_d[:])
            urow = constp.tile([1, rows], F16)
            nc.sync.dma_start(urow[:], u_d[:])
            cpk32 = constp.tile([128, CPK32_W], F32)
            nc.sync.dma_start(cpk32[:], cpk32_d[:])
            lpack = constp.tile([128, 2], U32)
            nc.sync.dma_start(lpack[:], lpack_d[:])

            cT16 = cpk16[:, P16_CT : P16_CT + 512]
            w2neg16_row = cpk16[0:1, P16_W2 : P16_W2 + 128]
            ident32 = cpk32[:, P32_IDENT : P32_IDENT + 128]
            cdv_col = cpk32[:, P32_CDV : P32_CDV + 1]
            mask_col = lpack[:, LP_MASK : LP_MASK + 1]
            lidx_col = lpack[:, LP_LIDX : LP_LIDX + 1]
            idxacc = outp.tile([128, ntiles, 8], U32)

            # warmup: make each engine observe a const-DMA lane with a
            # single-wait instruction, so later instructions carry at most
            # one new wait each.
            warm_ps = warmp.tile([128, 128], F32, tag="warm_ps")
            nc.tensor.transpose(warm_ps[:], ident32, ident32)
            warm16_ps = warmp.tile([1, 128], F16, tag="warm16_ps")
            nc.tensor.transpose(warm16_ps[:], cpk16[:, 0:1], cpk16[:, 0:128])
            act_warm = smallp.tile([1, 1], F32, tag="act_warm")
            nc.scalar.copy(act_warm[:], cpk32[0:1, 0:1])
            dve_warm = smallp.tile([1, 1], F16, tag="dve_warm")
            nc.vector.tensor_copy(dve_warm[:], urow[0:1, 0:1])
            gps_warm = smallp.tile([1, 1], U32, tag="gps_warm")
            nc.gpsimd.tensor_copy(gps_warm[:], lpack[0:1, 0:1])

            # first two groups load individually (halves pipeline-fill
            # latency); the rest in DMA_GROUPS batches, alternating queues
            sched = [(0, 1), (1, 1)] + [
                (gs, DMA_GROUPS) for gs in range(2, ngroups, DMA_GROUPS)
            ]
            for di, (gstart, gcnt) in enumerate(sched):
                ft = finp.tile([128, gcnt, 4, 512], F16, tag=f"ft{gcnt}")
                src = ft_d[:, gstart : gstart + gcnt, :]
                if di % 2 == 0:
                    nc.gpsimd.dma_start(ft[:], src)
                else:
                    nc.sync.dma_start(ft[:], src)

                for h in range(gcnt):
                    g = gstart + h
                    G_ps = gpp.tile([128, 512], F32)
                    for k in range(4):
                        nc.tensor.matmul(
                            G_ps[:],
                            cT16[:, k * 128 : (k + 1) * 128],
                            ft[:, h, k, :],
                            start=(k == 0),
                            stop=False,
                        )
                    nc.tensor.matmul(
                        G_ps[:],
                        w2neg16_row,
                        urow[0:1, g * 512 : (g + 1) * 512],
                        start=False,
                        stop=True,
                    )
                    # PSUM -> SBUF with exact f32 per-l constant (+SHIFT)
                    Gs = gsp.tile([128, 512], F32)
                    nc.scalar.activation(
                        Gs[:], G_ps[:], AF.Identity, bias=cdv_col, scale=1.0
                    )
                    # encode (score | 127-l) in u32 bits; partition == l here
                    S = encp.tile([128, 512], U32)
                    nc.vector.tensor_scalar(
                        S[:],
                        Gs[:].bitcast(U32),
                        mask_col,
                        lidx_col,
                        op0=ALU.bitwise_and,
                        op1=ALU.bitwise_or,
                    )
                    T_ps = tpp.tile([128, 512], F32)
                    Sf = S[:].bitcast(F32)
                    for j in range(4):
                        nc.tensor.transpose(
                            T_ps[:, j * 128 : (j + 1) * 128],
                            Sf[:, j * 128 : (j + 1) * 128],
                            ident32,
                        )
                    # MAX8 must run in f32: the u32 path mangles low mantissa
                    # bits (HW-verified), f32 is bit-exact and orders the same
                    # for these positive normal floats.
                    for j in range(4):
                        nc.vector.max(
                            idxacc[:, g * 4 + j, :].bitcast(F32),
                            T_ps[:, j * 128 : (j + 1) * 128],
                        )

                if g % 16 == 15:
                    t0 = (g - 15) * 4
                    nc.sync.dma_start(
                        idx_d[:, t0 : t0 + 64, :], idxacc[:, t0 : t0 + 64, :]
                    )
    if split_waits:
        _split_multiwait(nc)
    return nc


def make_const_inputs(initc, labelset):
    c = np.asarray(initc, dtype=np.float32)[np.asarray(labelset).astype(np.int64)]
    assert c.shape == (K, D + 1)
    w2 = 0.5 * np.sum(c.astype(np.float64) ** 2, axis=1)
    # Centering w2 (and shifting m by MBAR) adds per-row constants to the
    # scores (argmax invariant) while keeping the fp16-rounded rank-1 term
    # u*w2c small. The per-l constant cdv ships exact as an f32 ACT bias.
    w2c = w2 - w2.mean()
    cdv = (c[:, D].astype(np.float64) - MBAR * w2c + SHIFT).astype(np.float32)

    cpk16 = np.zeros((128, CPK16_W), np.float16)
    for k in range(4):
        cpk16[:, P16_CT + k * 128 : P16_CT + (k + 1) * 128] = (
            c[:, k * 128 : (k + 1) * 128].T.astype(np.float16)
        )
    cpk16[0, P16_W2 : P16_W2 + 128] = (-w2c).astype(np.float16)

    cpk32 = np.zeros((128, CPK32_W), np.float32)
    cpk32[:, P32_IDENT : P32_IDENT + 128] = np.eye(128, dtype=np.float32)
    cpk32[:, P32_CDV] = cdv

    lpack = np.zeros((128, 2), np.uint32)
    lpack[:, LP_MASK] = np.uint32(ENC_MASK)
    lpack[:, LP_LIDX] = (127 - np.arange(128)).astype(np.uint32)
    return {"cpk16": cpk16, "cpk32": cpk32, "lpack": lpack}


def pack_feats_core(shard16):
    """[32768, 512] fp16 -> [128, 64, 2048]: P[p, g, k*512+n] = s[g*512+n, k*128+p]."""
    ngroups = shard16.shape[0] // GROUP
    P = shard16.reshape(ngroups, GROUP, 4, 128).transpose(3, 0, 2, 1)
    return np.ascontiguousarray(P).reshape(128, ngroups, 2048)


def build_in_maps(feats, initc, labelset):
    feats = np.asarray(feats, dtype=np.float32)
    consts = make_const_inputs(initc, labelset)

    feats16 = feats.astype(np.float16)
    q = np.einsum("nd,nd->n", feats, feats, dtype=np.float64)
    u16 = (np.sqrt(q + 1.0) - MBAR).astype(np.float16)

    in_maps = []
    for core in range(N_CORES):
        sl = slice(core * ROWS_PER_CORE, (core + 1) * ROWS_PER_CORE)
        in_maps.append(
            {
                "ftpack": pack_feats_core(feats16[sl]),
                "upack": np.ascontiguousarray(u16[sl].reshape(1, -1)),
                **consts,
            }
        )
    return in_maps


def kernel(feats, initc, labelset):
    labelset_np = np.asarray(labelset)
    in_maps = build_in_maps(feats, initc, labelset)
    nc = build_core_program(ROWS_PER_CORE)
    res = run_bass_kernel_spmd(nc, in_maps, list(range(N_CORES)))

    preds = []
    for core in range(N_CORES):
        idx8 = np.asarray(res.results[core]["idx8"])  # [128, ntiles, 8]
        enc = idx8[:, :, 0]
        idx = (127 - (enc & np.uint32(127))).astype(np.int64)
        preds.append(idx.T.reshape(-1))  # row 128*t+p
    idx_all = np.concatenate(preds)
    return labelset_np[idx_all]


# revision 14
# speedup vs baseline: 1.3546x; 1.0155x over previous
"""Trainium2 Bass kernel: nearest-centroid assignment (vq_codebook).

Computes, for each row of `feats` [N, 512]:
    f = normalize([feats_n, 1])            (L2, with appended ones column)
    pred_n = labelset[argmin_l ||f - c_l||]   with c = initc[labelset]  [128, 513]

Equivalent argmax formulation used on device (monotone transform, per row n):
    argmin_l dist  ==  argmax_l  T[n, l]
    T[n, l] = sum_d feats[n,d] * c[l,d]  +  cdv_l  +  u_n * w2neg_l
    cdv_l   = c[l,512] - MBAR * w2c_l      (per-l constant, exact f32)
    u_n     = sqrt(|feats_n|^2 + 1) - MBAR
    w2neg_l = -(0.5*|c_l|^2 - mean)        (centered, fp16)

Sharding: pure data-parallel over rows, N/8 = 32768 rows per NeuronCore.

v3 dataflow: host pre-packs feats in fp16 *transposed* layout
    ftpack[p, g, k*512 + n] = feats[g*512 + n, k*128 + p]
so the DMA streams matmul-ready fT tiles (contiguous 8+ KB per partition
line per transfer); no on-device transposition of feats is needed. The
per-row norm term u is host-computed (O(N*D) preprocessing, same class
as the centroid cpack) and shipped as a [1, 32768] fp16 row. HBM
traffic per core: 32 MiB fp16.

Argmax via u32 score|index encoding (avoids the 3-instruction
MAX8/MATCH_VALUE_LOAD/FIND_INDEX8 chain per 128-row subtile):
    scores shifted positive (+4096 folded into the ACT bias), so their
    f32 bit patterns order like the values as unsigned ints; GPSIMD
    clears the low 7 mantissa bits (quantization ~0.06 abs, far below
    typical top-2 gaps) and ORs in (127 - l) -- both per-partition
    scalars in G-layout where partition == l. PE transposes the encoded
    bits (pass-through for normal floats); a single MAX8 per subtile
    then yields max-and-argmax in one op; host decodes l = 127 - (v&127).
    Ties resolve to the smallest l, matching argmin's first-index rule.

Per-core dataflow (64 groups of 512 rows):
    DMA    : 1 MiB fT tile per 2 groups, alternating gpsimd/sync queues
    PE     : 4x matmul  G[l,n] += cT16_k.T @ fT_k      (fp16, N=512)
             1x matmul  G[l,n] += w2neg16.T @ u_row     (K=1 rank-1)
    ACT    : G_ps -> SBUF copy with bias = cdv + 4096  (exact f32)
    GPSIMD : encode S = (Gs & 0xFFFFFF80) | (127-l)     (tensor_scalar)
    PE     : 4x transpose 128x128 of encoded bits: S -> T_enc[n, l]
    DVE    : 1x MAX8 per subtile directly from PSUM -> idxacc

Notes on this walrus build:
  - one sync wait per engine instruction: residual multi-wait
    instructions are split via same-engine NoOps (_split_multiwait).
  - all matmuls in the G accumulation group are fp16 (mixing fp16 and
    fp32r in one PSUM group silently corrupts on hardware).
"""

import os
import sys

import numpy as np

for _p in ("/opt/trn_rl_repo",):
    if _p not in sys.path and os.path.isdir(_p):
        sys.path.insert(0, _p)

import concourse.bass as bass
import concourse.mybir as mybir
import concourse.tile as tile
from concourse.bass_utils import run_bass_kernel_spmd

N, D, K = 262144, 512, 128
N_CORES = 8
ROWS_PER_CORE = N // N_CORES  # 32768
GROUP = 512  # rows per compute group (4 subtiles of 128)
DMA_GROUPS = 2  # groups per DMA (1 MiB transfers)

F32 = mybir.dt.float32
F16 = mybir.dt.float16
U32 = mybir.dt.uint32
AF = mybir.ActivationFunctionType
ALU = mybir.AluOpType

# fp16 constant pack [128, CPK16_W]
P16_CT = 0         # [128, 512] cT: cT[p, 128k+l] = c[l, 128k+p]
P16_W2 = 512       # row 0, [1, 128] w2neg = -(w2 - mean(w2))
CPK16_W = 640

# f32 constant pack [128, CPK32_W]
P32_IDENT = 0      # [128, 128] identity
P32_CDV = 128      # [128, 1] column: cdv_l + SHIFT
CPK32_W = 130

# u32 constant pack [128, 2]: col 0 = mantissa mask, col 1 = 127 - l
LP_MASK = 0
LP_LIDX = 1

MBAR = float(np.sqrt(513.0))  # fixed shift for m; argmax-invariant
SHIFT = 4096.0                # makes scores positive (u32-orderable bits)
ENC_MASK = 0xFFFFFF80         # clear low 7 mantissa bits for the l index


def _split_multiwait(nc):
    """Walrus (this build) allows one sync wait per engine instruction.

    Tile occasionally emits 2+ (data dep + buffer-slot release on another
    semaphore). Splitting is semantics-preserving: a same-engine NoOp placed
    immediately before the instruction carries the surplus waits; the engine
    executes in order, so all waits are still enforced before the instruction
    runs.
    """
    import bass_rust

    for fn in nc.m.functions:
        for blk in fn.blocks:
            out = []
            changed = False
            for inst in blk.instructions:
                si = getattr(inst, "sync_info", None)
                waits = list(si.on_wait) if si is not None else []
                if len(waits) > 1:
                    for w in waits[:-1]:
                        nop = mybir.InstNoOp(
                            name=nc.get_next_instruction_name(), ins=[], outs=[]
                        )
                        nop.engine = inst.engine
                        nop.sync_info = bass_rust.SyncInfo(
                            on_wait=[w], on_update=[]
                        )
                        out.append(nop)
                    inst.sync_info = bass_rust.SyncInfo(
                        on_wait=waits[-1:], on_update=list(si.on_update)
                    )
                    changed = True
                out.append(inst)
            if changed:
                blk.instructions = out


def build_core_program(rows=ROWS_PER_CORE, split_waits=True):
    """Bass program for one NeuronCore processing `rows` rows of feats."""
    assert rows % (GROUP * DMA_GROUPS) == 0
    ngroups = rows // GROUP           # 64
    ndma = ngroups // DMA_GROUPS      # 32
    ntiles = rows // 128              # 256

    nc = bass.Bass()
    ft_d = nc.declare_dram_parameter("ftpack", [128, ngroups, 2048], F16, isOutput=False)
    cpk16_d = nc.declare_dram_parameter("cpk16", [128, CPK16_W], F16, isOutput=False)
    cpk32_d = nc.declare_dram_parameter("cpk32", [128, CPK32_W], F32, isOutput=False)
    lpack_d = nc.declare_dram_parameter("lpack", [128, 2], U32, isOutput=False)
    u_d = nc.declare_dram_parameter("upack", [1, rows], F16, isOutput=False)
    idx_d = nc.declare_dram_parameter("idx8", [128, ntiles, 8], U32, isOutput=True)

    with tile.TileContext(nc) as tc:
        with (
            tc.tile_pool(name="const", bufs=1) as constp,
            tc.tile_pool(name="fin", bufs=3) as finp,
            tc.tile_pool(name="gs", bufs=2) as gsp,
            tc.tile_pool(name="enc", bufs=2) as encp,
            tc.tile_pool(name="small", bufs=1) as smallp,
            tc.tile_pool(name="outp", bufs=1) as outp,
            tc.tile_pool(name="gp", bufs=2, space="PSUM") as gpp,
            tc.tile_pool(name="tp", bufs=2, space="PSUM") as tpp,
        ):
            cpk16 = constp.tile([128, CPK16_W], F16)
            nc.sync.dma_start(cpk16[:], cpk16_d[:])
            urow = constp.tile([1, rows], F16)
            nc.sync.dma_start(urow[:], u_d[:])
            cpk32 = constp.tile([128, CPK32_W], F32)
            nc.sync.dma_start(cpk32[:], cpk32_d[:])
            lpack = constp.tile([128, 2], U32)
            nc.sync.dma_start(lpack[:], lpack_d[:])

            cT16 = cpk16[:, P16_CT : P16_CT + 512]
            w2neg16_row = cpk16[0:1, P16_W2 : P16_W2 + 128]
            ident32 = cpk32[:, P32_IDENT : P32_IDENT + 128]
            cdv_col = cpk32[:, P32_CDV : P32_CDV + 1]
            mask_col = lpack[:, LP_MASK : LP_MASK + 1]
            lidx_col = lpack[:, LP_LIDX : LP_LIDX + 1]
            idxacc = outp.tile([128, ntiles, 8], U32)

            # warmup: make ACT/DVE observe their const-DMA lanes with a
            # single-wait instruction each; residual multi-waits elsewhere
            # are handled by _split_multiwait.
            act_warm = smallp.tile([1, 1], F32, tag="act_warm")
            nc.scalar.copy(act_warm[:], cpk32[0:1, 0:1])
            dve_warm = smallp.tile([1, 1], F16, tag="dve_warm")
            nc.vector.tensor_copy(dve_warm[:], urow[0:1, 0:1])

            # feature loads spread over three DMA queues (SWDGE + both HWDGE)
            for dg in range(ndma):
                ft = finp.tile([128, DMA_GROUPS, 4, 512], F16)
                src = ft_d[:, dg * DMA_GROUPS : (dg + 1) * DMA_GROUPS, :]
                queue = (nc.gpsimd, nc.sync, nc.scalar)[dg % 3]
                queue.dma_start(ft[:], src)

                for h in range(DMA_GROUPS):
                    g = dg * DMA_GROUPS + h
                    G_ps = gpp.tile([128, 512], F32)
                    for k in range(4):
                        nc.tensor.matmul(
                            G_ps[:],
                            cT16[:, k * 128 : (k + 1) * 128],
                            ft[:, h, k, :],
                            start=(k == 0),
                            stop=False,
                        )
                    nc.tensor.matmul(
                        G_ps[:],
                        w2neg16_row,
                        urow[0:1, g * 512 : (g + 1) * 512],
                        start=False,
                        stop=True,
                    )
                    # PSUM -> SBUF with exact f32 per-l constant (+SHIFT)
                    Gs = gsp.tile([128, 512], F32)
                    nc.scalar.activation(
                        Gs[:], G_ps[:], AF.Identity, bias=cdv_col, scale=1.0
                    )
                    # encode (score | 127-l) in u32 bits; partition == l here
                    S = encp.tile([128, 512], U32)
                    nc.vector.tensor_scalar(
                        S[:],
                        Gs[:].bitcast(U32),
                        mask_col,
                        lidx_col,
                        op0=ALU.bitwise_and,
                        op1=ALU.bitwise_or,
                    )
                    T_ps = tpp.tile([128, 512], F32)
                    Sf = S[:].bitcast(F32)
                    for j in range(4):
                        nc.tensor.transpose(
                            T_ps[:, j * 128 : (j + 1) * 128],
                            Sf[:, j * 128 : (j + 1) * 128],
                            ident32,
                        )
                    # MAX8 must run in f32: the u32 path mangles low mantissa
                    # bits (HW-verified), f32 is bit-exact and orders the same
                    # for these positive normal floats.
                    for j in range(4):
                        nc.vector.max(
                            idxacc[:, g * 4 + j, :].bitcast(F32),
                            T_ps[:, j * 128 : (j + 1) * 128],
                        )

                if g % 16 == 15:
                    t0 = (g - 15) * 4
                    nc.scalar.dma_start(
                        idx_d[:, t0 : t0 + 64, :], idxacc[:, t0 : t0 + 64, :]
                    )
    if split_waits:
        _split_multiwait(nc)
    return nc


def make_const_inputs(initc, labelset):
    c = np.asarray(initc, dtype=np.float32)[np.asarray(labelset).astype(np.int64)]
    assert c.shape == (K, D + 1)
    w2 = 0.5 * np.sum(c.astype(np.float64) ** 2, axis=1)
    # Centering w2 (and shifting m by MBAR) adds per-row constants to the
    # scores (argmax invariant) while keeping the fp16-rounded rank-1 term
    # u*w2c small. The per-l constant cdv ships exact as an f32 ACT bias.
    w2c = w2 - w2.mean()
    cdv = (c[:, D].astype(np.float64) - MBAR * w2c + SHIFT).astype(np.float32)

    cpk16 = np.zeros((128, CPK16_W), np.float16)
    for k in range(4):
        cpk16[:, P16_CT + k * 128 : P16_CT + (k + 1) * 128] = (
            c[:, k * 128 : (k + 1) * 128].T.astype(np.float16)
        )
    cpk16[0, P16_W2 : P16_W2 + 128] = (-w2c).astype(np.float16)

    cpk32 = np.zeros((128, CPK32_W), np.float32)
    cpk32[:, P32_IDENT : P32_IDENT + 128] = np.eye(128, dtype=np.float32)
    cpk32[:, P32_CDV] = cdv

    lpack = np.zeros((128, 2), np.uint32)
    lpack[:, LP_MASK] = np.uint32(ENC_MASK)
    lpack[:, LP_LIDX] = (127 - np.arange(128)).astype(np.uint32)
    return {"cpk16": cpk16, "cpk32": cpk32, "lpack": lpack}


def pack_feats_core(shard16):
    """[32768, 512] fp16 -> [128, 64, 2048]: P[p, g, k*512+n] = s[g*512+n, k*128+p]."""
    ngroups = shard16.shape[0] // GROUP
    P = shard16.reshape(ngroups, GROUP, 4, 128).transpose(3, 0, 2, 1)
    return np.ascontiguousarray(P).reshape(128, ngroups, 2048)


def build_in_maps(feats, initc, labelset):
    feats = np.asarray(feats, dtype=np.float32)
    consts = make_const_inputs(initc, labelset)

    feats16 = feats.astype(np.float16)
    q = np.einsum("nd,nd->n", feats, feats, dtype=np.float64)
    u16 = (np.sqrt(q + 1.0) - MBAR).astype(np.float16)

    in_maps = []
    for core in range(N_CORES):
        sl = slice(core * ROWS_PER_CORE, (core + 1) * ROWS_PER_CORE)
        in_maps.append(
            {
                "ftpack": pack_feats_core(feats16[sl]),
                "upack": np.ascontiguousarray(u16[sl].reshape(1, -1)),
                **consts,
            }
        )
    return in_maps


def kernel(feats, initc, labelset):
    labelset_np = np.asarray(labelset)
    in_maps = build_in_maps(feats, initc, labelset)
    nc = build_core_program(ROWS_PER_CORE)
    res = run_bass_kernel_spmd(nc, in_maps, list(range(N_CORES)))

    preds = []
    for core in range(N_CORES):
        idx8 = np.asarray(res.results[core]["idx8"])  # [128, ntiles, 8]
        enc = idx8[:, :, 0]
        idx = (127 - (enc & np.uint32(127))).astype(np.int64)
        preds.append(idx.T.reshape(-1))  # row 128*t+p
    idx_all = np.concatenate(preds)
    return labelset_np[idx_all]


# revision 16
# speedup vs baseline: 1.3599x; 1.0039x over previous
"""Trainium2 Bass kernel: nearest-centroid assignment (vq_codebook).

Computes, for each row of `feats` [N, 512]:
    f = normalize([feats_n, 1])            (L2, with appended ones column)
    pred_n = labelset[argmin_l ||f - c_l||]   with c = initc[labelset]  [128, 513]

Equivalent argmax formulation used on device (monotone transform, per row n):
    argmin_l dist  ==  argmax_l  T[n, l]
    T[n, l] = sum_d feats[n,d] * c[l,d]  +  cdv_l  +  u_n * w2neg_l
    cdv_l   = c[l,512] - MBAR * w2c_l      (per-l constant, exact f32)
    u_n     = sqrt(|feats_n|^2 + 1) - MBAR
    w2neg_l = -(0.5*|c_l|^2 - mean)        (centered, fp16)

Sharding: pure data-parallel over rows, N/8 = 32768 rows per NeuronCore.

v3 dataflow: host pre-packs feats in fp16 *transposed* layout
    ftpack[p, g, k*512 + n] = feats[g*512 + n, k*128 + p]
so the DMA streams matmul-ready fT tiles (contiguous 8+ KB per partition
line per transfer); no on-device transposition of feats is needed. The
per-row norm term u is host-computed (O(N*D) preprocessing, same class
as the centroid cpack) and shipped as a [1, 32768] fp16 row. HBM
traffic per core: 32 MiB fp16.

Argmax via u32 score|index encoding (avoids the 3-instruction
MAX8/MATCH_VALUE_LOAD/FIND_INDEX8 chain per 128-row subtile):
    scores shifted positive (+4096 folded into the ACT bias), so their
    f32 bit patterns order like the values as unsigned ints; GPSIMD
    clears the low 7 mantissa bits (quantization ~0.06 abs, far below
    typical top-2 gaps) and ORs in (127 - l) -- both per-partition
    scalars in G-layout where partition == l. PE transposes the encoded
    bits (pass-through for normal floats); a single MAX8 per subtile
    then yields max-and-argmax in one op; host decodes l = 127 - (v&127).
    Ties resolve to the smallest l, matching argmin's first-index rule.

Per-core dataflow (64 groups of 512 rows):
    DMA    : 1 MiB fT tile per 2 groups, alternating gpsimd/sync queues
    PE     : 4x matmul  G[l,n] += cT16_k.T @ fT_k      (fp16, N=512)
             1x matmul  G[l,n] += w2neg16.T @ u_row     (K=1 rank-1)
    ACT    : G_ps -> SBUF copy with bias = cdv + 4096  (exact f32)
    GPSIMD : encode S = (Gs & 0xFFFFFF80) | (127-l)     (tensor_scalar)
    PE     : 4x transpose 128x128 of encoded bits: S -> T_enc[n, l]
    DVE    : 1x MAX8 per subtile directly from PSUM -> idxacc

Notes on this walrus build:
  - one sync wait per engine instruction: residual multi-wait
    instructions are split via same-engine NoOps (_split_multiwait).
  - all matmuls in the G accumulation group are fp16 (mixing fp16 and
    fp32r in one PSUM group silently corrupts on hardware).
"""

import os
import sys

import numpy as np

for _p in ("/opt/trn_rl_repo",):
    if _p not in sys.path and os.path.isdir(_p):
        sys.path.insert(0, _p)

import concourse.bass as bass
import concourse.mybir as mybir
import concourse.tile as tile
from concourse.bass_utils import run_bass_kernel_spmd

N, D, K = 262144, 512, 128
N_CORES = 8
ROWS_PER_CORE = N // N_CORES  # 32768
GROUP = 512  # rows per compute group (4 subtiles of 128)
DMA_GROUPS = 2  # groups per DMA (1 MiB transfers)

F32 = mybir.dt.float32
F16 = mybir.dt.float16
U32 = mybir.dt.uint32
AF = mybir.ActivationFunctionType
ALU = mybir.AluOpType

# fp16 constant pack [128, CPK16_W]
P16_CT = 0         # [128, 512] cT: cT[p, 128k+l] = c[l, 128k+p]
P16_W2 = 512       # row 0, [1, 128] w2neg = -(w2 - mean(w2))
CPK16_W = 640

# f32 constant pack [128, CPK32_W]
P32_IDENT = 0      # [128, 128] identity
P32_CDV = 128      # [128, 1] column: cdv_l + SHIFT
CPK32_W = 130

# u32 constant pack [128, 2]: col 0 = mantissa mask, col 1 = 127 - l
LP_MASK = 0
LP_LIDX = 1

MBAR = float(np.sqrt(513.0))  # fixed shift for m; argmax-invariant
SHIFT = 4096.0                # makes scores positive (u32-orderable bits)
ENC_MASK = 0xFFFFFF80         # clear low 7 mantissa bits for the l index


def _split_multiwait(nc):
    """Walrus (this build) allows one sync wait per engine instruction.

    Tile occasionally emits 2+ (data dep + buffer-slot release on another
    semaphore). Splitting is semantics-preserving: a same-engine NoOp placed
    immediately before the instruction carries the surplus waits; the engine
    executes in order, so all waits are still enforced before the instruction
    runs.
    """
    import bass_rust

    for fn in nc.m.functions:
        for blk in fn.blocks:
            out = []
            changed = False
            for inst in blk.instructions:
                si = getattr(inst, "sync_info", None)
                waits = list(si.on_wait) if si is not None else []
                if len(waits) > 1:
                    for w in waits[:-1]:
                        nop = mybir.InstNoOp(
                            name=nc.get_next_instruction_name(), ins=[], outs=[]
                        )
                        nop.engine = inst.engine
                        nop.sync_info = bass_rust.SyncInfo(
                            on_wait=[w], on_update=[]
                        )
                        out.append(nop)
                    inst.sync_info = bass_rust.SyncInfo(
                        on_wait=waits[-1:], on_update=list(si.on_update)
                    )
                    changed = True
                out.append(inst)
            if changed:
                blk.instructions = out


def build_core_program(rows=ROWS_PER_CORE, split_waits=True):
    """Bass program for one NeuronCore processing `rows` rows of feats."""
    assert rows % (GROUP * DMA_GROUPS) == 0
    ngroups = rows // GROUP           # 64
    ndma = ngroups // DMA_GROUPS      # 32
    ntiles = rows // 128              # 256

    nc = bass.Bass()
    ft_d = nc.declare_dram_parameter("ftpack", [128, ngroups, 2048], F16, isOutput=False)
    cpk16_d = nc.declare_dram_parameter("cpk16", [128, CPK16_W], F16, isOutput=False)
    cpk32_d = nc.declare_dram_parameter("cpk32", [128, CPK32_W], F32, isOutput=False)
    lpack_d = nc.declare_dram_parameter("lpack", [128, 2], U32, isOutput=False)
    u_d = nc.declare_dram_parameter("upack", [1, rows], F16, isOutput=False)
    idx_d = nc.declare_dram_parameter("idx8", [128, ntiles, 8], U32, isOutput=True)

    with tile.TileContext(nc) as tc:
        with (
            tc.tile_pool(name="const", bufs=1) as constp,
            tc.tile_pool(name="fin", bufs=3) as finp,
            tc.tile_pool(name="gs", bufs=2) as gsp,
            tc.tile_pool(name="enc", bufs=2) as encp,
            tc.tile_pool(name="small", bufs=1) as smallp,
            tc.tile_pool(name="outp", bufs=1) as outp,
            tc.tile_pool(name="gp", bufs=2, space="PSUM") as gpp,
            tc.tile_pool(name="tp", bufs=2, space="PSUM") as tpp,
        ):
            # constants ride the scalar HWDGE queue; its first feature load
            # (dg=2) isn't needed until ~7us in, so they never delay features
            cpk16 = constp.tile([128, CPK16_W], F16)
            nc.scalar.dma_start(cpk16[:], cpk16_d[:])
            urow = constp.tile([1, rows], F16)
            nc.scalar.dma_start(urow[:], u_d[:])
            cpk32 = constp.tile([128, CPK32_W], F32)
            nc.scalar.dma_start(cpk32[:], cpk32_d[:])
            lpack = constp.tile([128, 2], U32)
            nc.scalar.dma_start(lpack[:], lpack_d[:])

            cT16 = cpk16[:, P16_CT : P16_CT + 512]
            w2neg16_row = cpk16[0:1, P16_W2 : P16_W2 + 128]
            ident32 = cpk32[:, P32_IDENT : P32_IDENT + 128]
            cdv_col = cpk32[:, P32_CDV : P32_CDV + 1]
            mask_col = lpack[:, LP_MASK : LP_MASK + 1]
            lidx_col = lpack[:, LP_LIDX : LP_LIDX + 1]
            idxacc = outp.tile([128, ntiles, 8], U32)

            # warmup: make ACT/DVE observe their const-DMA lanes with a
            # single-wait instruction each; residual multi-waits elsewhere
            # are handled by _split_multiwait.
            act_warm = smallp.tile([1, 1], F32, tag="act_warm")
            nc.scalar.copy(act_warm[:], cpk32[0:1, 0:1])
            dve_warm = smallp.tile([1, 1], F16, tag="dve_warm")
            nc.vector.tensor_copy(dve_warm[:], urow[0:1, 0:1])

            # feature loads spread over three DMA queues (SWDGE + both HWDGE)
            for dg in range(ndma):
                ft = finp.tile([128, DMA_GROUPS, 4, 512], F16)
                src = ft_d[:, dg * DMA_GROUPS : (dg + 1) * DMA_GROUPS, :]
                queue = (nc.sync, nc.gpsimd, nc.scalar)[dg % 3]
                queue.dma_start(ft[:], src)

                for h in range(DMA_GROUPS):
                    g = dg * DMA_GROUPS + h
                    G_ps = gpp.tile([128, 512], F32)
                    for k in range(4):
                        nc.tensor.matmul(
                            G_ps[:],
                            cT16[:, k * 128 : (k + 1) * 128],
                            ft[:, h, k, :],
                            start=(k == 0),
                            stop=False,
                        )
                    nc.tensor.matmul(
                        G_ps[:],
                        w2neg16_row,
                        urow[0:1, g * 512 : (g + 1) * 512],
                        start=False,
                        stop=True,
                    )
                    # PSUM -> SBUF with exact f32 per-l constant (+SHIFT)
                    Gs = gsp.tile([128, 512], F32)
                    nc.scalar.activation(
                        Gs[:], G_ps[:], AF.Identity, bias=cdv_col, scale=1.0
                    )
                    # encode (score | 127-l) in u32 bits; partition == l here
                    S = encp.tile([128, 512], U32)
                    nc.vector.tensor_scalar(
                        S[:],
                        Gs[:].bitcast(U32),
                        mask_col,
                        lidx_col,
                        op0=ALU.bitwise_and,
                        op1=ALU.bitwise_or,
                    )
                    T_ps = tpp.tile([128, 512], F32)
                    Sf = S[:].bitcast(F32)
                    for j in range(4):
                        nc.tensor.transpose(
                            T_ps[:, j * 128 : (j + 1) * 128],
                            Sf[:, j * 128 : (j + 1) * 128],
                            ident32,
                        )
                    # MAX8 must run in f32: the u32 path mangles low mantissa
                    # bits (HW-verified), f32 is bit-exact and orders the same
                    # for these positive normal floats.
                    for j in range(4):
                        nc.vector.max(
                            idxacc[:, g * 4 + j, :].bitcast(F32),
                            T_ps[:, j * 128 : (j + 1) * 128],
                        )

                if g % 16 == 15:
                    t0 = (g - 15) * 4
                    nc.scalar.dma_start(
                        idx_d[:, t0 : t0 + 64, :], idxacc[:, t0 : t0 + 64, :]
                    )
    if split_waits:
        _split_multiwait(nc)
    return nc


def make_const_inputs(initc, labelset):
    c = np.asarray(initc, dtype=np.float32)[np.asarray(labelset).astype(np.int64)]
    assert c.shape == (K, D + 1)
    w2 = 0.5 * np.sum(c.astype(np.float64) ** 2, axis=1)
    # Centering w2 (and shifting m by MBAR) adds per-row constants to the
    # scores (argmax invariant) while keeping the fp16-rounded rank-1 term
    # u*w2c small. The per-l constant cdv ships exact as an f32 ACT bias.
    w2c = w2 - w2.mean()
    cdv = (c[:, D].astype(np.float64) - MBAR * w2c + SHIFT).astype(np.float32)

    cpk16 = np.zeros((128, CPK16_W), np.float16)
    for k in range(4):
        cpk16[:, P16_CT + k * 128 : P16_CT + (k + 1) * 128] = (
            c[:, k * 128 : (k + 1) * 128].T.astype(np.float16)
        )
    cpk16[0, P16_W2 : P16_W2 + 128] = (-w2c).astype(np.float16)

    cpk32 = np.zeros((128, CPK32_W), np.float32)
    cpk32[:, P32_IDENT : P32_IDENT + 128] = np.eye(128, dtype=np.float32)
    cpk32[:, P32_CDV] = cdv

    lpack = np.zeros((128, 2), np.uint32)
    lpack[:, LP_MASK] = np.uint32(ENC_MASK)
    lpack[:, LP_LIDX] = (127 - np.arange(128)).astype(np.uint32)
    return {"cpk16": cpk16, "cpk32": cpk32, "lpack": lpack}


def pack_feats_core(shard16):
    """[32768, 512] fp16 -> [128, 64, 2048]: P[p, g, k*512+n] = s[g*512+n, k*128+p]."""
    ngroups = shard16.shape[0] // GROUP
    P = shard16.reshape(ngroups, GROUP, 4, 128).transpose(3, 0, 2, 1)
    return np.ascontiguousarray(P).reshape(128, ngroups, 2048)


def build_in_maps(feats, initc, labelset):
    feats = np.asarray(feats, dtype=np.float32)
    consts = make_const_inputs(initc, labelset)

    feats16 = feats.astype(np.float16)
    q = np.einsum("nd,nd->n", feats, feats, dtype=np.float64)
    u16 = (np.sqrt(q + 1.0) - MBAR).astype(np.float16)

    in_maps = []
    for core in range(N_CORES):
        sl = slice(core * ROWS_PER_CORE, (core + 1) * ROWS_PER_CORE)
        in_maps.append(
            {
                "ftpack": pack_feats_core(feats16[sl]),
                "upack": np.ascontiguousarray(u16[sl].reshape(1, -1)),
                **consts,
            }
        )
    return in_maps


def kernel(feats, initc, labelset):
    labelset_np = np.asarray(labelset)
    in_maps = build_in_maps(feats, initc, labelset)
    nc = build_core_program(ROWS_PER_CORE)
    res = run_bass_kernel_spmd(nc, in_maps, list(range(N_CORES)))

    preds = []
    for core in range(N_CORES):
        idx8 = np.asarray(res.results[core]["idx8"])  # [128, ntiles, 8]
        enc = idx8[:, :, 0]
        idx = (127 - (enc & np.uint32(127))).astype(np.int64)
        preds.append(idx.T.reshape(-1))  # row 128*t+p
    idx_all = np.concatenate(preds)
    return labelset_np[idx_all]
